# revision 23
# baseline (speedup 1.0000x reference)
"""Linear attention (B=4, S=4096, D=1024, H=16) on 8 TRN2 NeuronCores.

Sharding: core = (batch, head-half): each core handles one batch's 8 heads.
 - x is host-transposed to xT [D, S] per batch so both operand orientations
   of every matmul come out of the tensor engine with no on-device transpose.
 - Wqkv column-sharded per head-half; Wo row-sharded; host sums the two
   partial y's per batch (row-parallel unshard).

Two-phase dataflow (all matmuls bf16, fp32 PSUM accumulate):

phase 1 (per 512-token block): K,V projection token-major (lhsT=xT slice,
  rhs=Wk/Wv) -> elu+1(K) -> [KV | K_sum^T] PSUM accumulation per head-pair
  (vst carries a ones column so one matmul does both). Q is NOT computed
  here -- it is deferred to phase 2 so the PE has independent work to chew
  on across the KV -> attention transition (no pipeline bubble), and so x
  (kept resident in SBUF, 8MB bf16) is the only phase-1 input.
  Block 0 runs k-outer (4 simultaneous PSUM chains, one per 128-token
  subtile) so compute starts as soon as the first (wkv, x) DMA chunk lands
  instead of waiting for the full weight load.

phase 2 (per block, software-pipelined across j):
  QT [512f, 512s] feature-major (lhsT=Wq, rhs=xT slice) -> elu+1 -> bf16
  psc[128,s] = blockdiag(KV_h0, KV_h1)^T @ QT_pair: both heads of a pair
    in one matmul; ACT-evicted to outu
  norm: lhsT = [ksum_h0 replicated x64 | ksum_h1 replicated x64] so the
    matmul output IS the normalizer broadcast across all 128 partitions
    (no separate broadcast matmul); 1/x via the single-instruction DVE
    fast reciprocal (no Ln/Exp ACT ops, no activation-table switches)
  outT = outu * rcp (one DVE mult per pair, bf16)
  y[s,:] = outT^T @ Wo per 128-token subtile, fp32 out, DMAed per subtile
    (512KB chunks) to keep the drain tail short.
"""

import numpy as np

import concourse.bacc as bacc
import concourse.mybir as mybir
import concourse.tile as tile
from concourse.bass_utils import run_bass_kernel_spmd

F32 = mybir.dt.float32
BF16 = mybir.dt.bfloat16
ACT = mybir.ActivationFunctionType

P = 128
B, S, D = 4, 4096, 1024
H = 16
HD = 64

FSH = 512            # features per core for each of Q, K, V (8 heads)
KSUB = D // P        # 8 contraction subtiles
SBLK = 512           # tokens per block
NBLK = S // SBLK     # 8 blocks
TSUB = SBLK // P     # 4 token subtiles per block
NPAIR = 4            # head pairs per core

_NC_CACHE = None


def build():
    nc = bacc.Bacc(target_bir_lowering=False)
    xT = nc.dram_tensor("xT", [D, S], BF16, kind="ExternalInput")
    wqkv = nc.dram_tensor("wqkv", [D, 3 * FSH], BF16, kind="ExternalInput")
    wo = nc.dram_tensor("wo", [FSH, D], BF16, kind="ExternalInput")
    y = nc.dram_tensor("y", [S, D], F32, kind="ExternalOutput")

    xT_r = xT.rearrange("(ko p) s -> p ko s", p=P)        # [128, 8, 4096]
    wqkv_r = wqkv.rearrange("(ko p) f -> p ko f", p=P)    # [128, 8, 1536]
    wo_r = wo.rearrange("(fo p) n -> p fo n", p=P)        # [128, 4, 1024]
    y_rt = y.rearrange(
        "(j t p) (nh n) -> j t nh p n", t=TSUB, p=P, nh=2
    )  # [8,4,2,128,512]

    with tile.TileContext(nc) as tc:
        import contextlib

        with contextlib.ExitStack() as ctx:
            wpool = ctx.enter_context(tc.tile_pool(name="wpool", bufs=1))

            # persistent SBUF
            xt_sb = wpool.tile([P, KSUB, S], BF16)          # all of x, 64KB/p
            wqkv_sb = wpool.tile([P, KSUB, 3 * FSH], BF16)  # [wq|wk|wv]
            wo_sb = wpool.tile([P, FSH // P, D], BF16)
            # per-pair block-diagonal [[KV_h0, 0], [0, KV_h1]] (128x128)
            lhsT2_sb = [
                wpool.tile([P, P], BF16, name=f"l2{p}") for p in range(NPAIR)
            ]
            # per-pair [ksum_h0 x64 | ksum_h1 x64] replicated along free dim:
            # norm matmul output comes out already broadcast per head-half
            ksumrep_sb = [
                wpool.tile([P, P], BF16, name=f"kr{p}") for p in range(NPAIR)
            ]

            # Only block-0-critical transfers go on the sync queue (issued at
            # t=0): x block 0 + wv interleaved per contraction subtile, then
            # wk. Everything else (x blocks 1-7, wq, wo) is issued from the
            # scalar engine's DGE mid-block-0 — gated behind ops that already
            # depend on block-0 data — so it cannot steal HBM bandwidth from
            # the startup-critical path.
            for k in range(KSUB):
                nc.sync.dma_start(
                    out=xt_sb[:, k, 0:SBLK], in_=xT_r[:, k, 0:SBLK]
                )
                nc.sync.dma_start(
                    out=wqkv_sb[:, k, 2 * FSH : 3 * FSH],
                    in_=wqkv_r[:, k, 2 * FSH : 3 * FSH],
                )
            for k in range(KSUB):
                nc.sync.dma_start(
                    out=wqkv_sb[:, k, FSH : 2 * FSH],
                    in_=wqkv_r[:, k, FSH : 2 * FSH],
                )
            for p_ in range(NPAIR):
                nc.vector.memset(lhsT2_sb[p_], 0.0)
                nc.vector.memset(ksumrep_sb[p_], 0.0)

            # Non-critical DMAs queue behind the block-0 set on the same
            # hardware queue, so single-queue in-order dispatch gates them
            # without explicit dependencies.
            for j in range(1, NBLK):
                nc.sync.dma_start(
                    out=xt_sb[:, :, j * SBLK : (j + 1) * SBLK],
                    in_=xT_r[:, :, j * SBLK : (j + 1) * SBLK],
                )
            for k in range(KSUB):
                nc.sync.dma_start(
                    out=wqkv_sb[:, k, 0:FSH], in_=wqkv_r[:, k, 0:FSH]
                )
            nc.sync.dma_start(out=wo_sb, in_=wo_r)

            # SBUF pools shared across both phases
            etpool = ctx.enter_context(tc.tile_pool(name="et", bufs=3))
            qtpool = ctx.enter_context(tc.tile_pool(name="qt", bufs=2))
            qts = {}

            def qt_elu(ps, j, f):
                # elu(x)+1 = min(exp(x),1) + relu(x); Exp/Relu on ACT, the
                # combine on DVE
                e = etpool.tile([P, SBLK], F32, tag="e")
                nc.scalar.activation(out=e, in_=ps, func=ACT.Exp)
                r = etpool.tile([P, SBLK], F32, tag="r")
                nc.scalar.activation(out=r, in_=ps, func=ACT.Relu)
                nc.vector.scalar_tensor_tensor(
                    out=qts[j][:, f, :],
                    in0=e,
                    scalar=1.0,
                    in1=r,
                    op0=mybir.AluOpType.min,
                    op1=mybir.AluOpType.add,
                )

            # ---------------- phase 1: K,V projection + KV accumulation ----
            with (
                tc.tile_pool(name="kvps", bufs=1, space="PSUM") as kvps_pool,
                tc.tile_pool(name="pa", bufs=4, space="PSUM") as pa_pool,
                tc.tile_pool(name="st", bufs=2) as stpool,
            ):
                kvps = [
                    kvps_pool.tile([P, P + 1], F32, tag=f"kv{p}", name=f"kv{p}")
                    for p in range(NPAIR)
                ]

                bq = []  # lagged [KV | K_sum] accumulation entries

                def emit_b(ent):
                    kst, vst, j, t = ent
                    first = j == 0 and t == 0
                    last = j == NBLK - 1 and t == TSUB - 1
                    for p_ in range(NPAIR):
                        nc.tensor.matmul(
                            kvps[p_],
                            kst[:, t, p_ * P : (p_ + 1) * P],
                            vst[:, t, p_, :],
                            start=first,
                            stop=last,
                        )

                def elu_k(ps, kst, t):
                    e = etpool.tile([P, SBLK], F32, tag="e")
                    nc.scalar.activation(out=e, in_=ps, func=ACT.Exp)
                    r = etpool.tile([P, SBLK], F32, tag="r")
                    nc.vector.tensor_scalar_max(r, ps, 0.0)
                    nc.vector.scalar_tensor_tensor(
                        out=kst[:, t, :],
                        in0=e,
                        scalar=1.0,
                        in1=r,
                        op0=mybir.AluOpType.min,
                        op1=mybir.AluOpType.add,
                    )

                # block 0: V-sweep k-outer so PE work tracks DMA chunk
                # arrival (V needs no activation, so the K sweep that
                # follows runs at PE speed with elu pipelined per subtile)
                kst0 = stpool.tile([P, TSUB, FSH], BF16, tag="kst")
                vst0 = stpool.tile([P, TSUB, NPAIR, P + 1], BF16, tag="vst")
                nc.vector.memset(vst0[:, :, :, P : P + 1], 1.0)
                psvs = [
                    pa_pool.tile([P, SBLK], F32, tag="pa", name=f"psv{t}")
                    for t in range(TSUB)
                ]
                for k in range(KSUB):
                    for t in range(TSUB):
                        nc.tensor.matmul(
                            psvs[t],
                            xt_sb[:, k, t * P : (t + 1) * P],
                            wqkv_sb[:, k, 2 * FSH : 3 * FSH],
                            start=(k == 0),
                            stop=(k == KSUB - 1),
                        )
                for t in range(TSUB):
                    nc.scalar.copy(out=vst0[:, t, :, 0:P], in_=psvs[t])
                for t in range(TSUB):
                    psk = pa_pool.tile([P, SBLK], F32, tag="pa", name=f"psk{t}")
                    for k in range(KSUB):
                        nc.tensor.matmul(
                            psk,
                            xt_sb[:, k, t * P : (t + 1) * P],
                            wqkv_sb[:, k, FSH : 2 * FSH],
                            start=(k == 0),
                            stop=(k == KSUB - 1),
                        )
                    if t >= 1:
                        emit_b(bq.pop(0))
                    elu_k(psk, kst0, t)
                    bq.append((kst0, vst0, 0, t))

                # blocks 1..7: token-subtile-outer, B lagged one step
                for j in range(1, NBLK):
                    kst = stpool.tile([P, TSUB, FSH], BF16, tag="kst")
                    vst = stpool.tile([P, TSUB, NPAIR, P + 1], BF16, tag="vst")
                    nc.vector.memset(vst[:, :, :, P : P + 1], 1.0)
                    for t in range(TSUB):
                        tok = j * SBLK + t * P
                        psk = pa_pool.tile([P, SBLK], F32, tag="pa")
                        psv = pa_pool.tile([P, SBLK], F32, tag="pa")
                        for k in range(KSUB):
                            nc.tensor.matmul(
                                psk,
                                xt_sb[:, k, tok : tok + P],
                                wqkv_sb[:, k, FSH : 2 * FSH],
                                start=(k == 0),
                                stop=(k == KSUB - 1),
                            )
                            nc.tensor.matmul(
                                psv,
                                xt_sb[:, k, tok : tok + P],
                                wqkv_sb[:, k, 2 * FSH : 3 * FSH],
                                start=(k == 0),
                                stop=(k == KSUB - 1),
                            )
                        emit_b(bq.pop(0))
                        elu_k(psk, kst, t)
                        nc.scalar.copy(out=vst[:, t, :, 0:P], in_=psv)
                        bq.append((kst, vst, j, t))
                # block 0's Q projection runs here, inside the phase-1 PSUM
                # pools: it has no dependency on the KV state, so it keeps
                # the PE busy across the phase boundary (the trailing elu,
                # the KV extraction, and the phase-2 pool handover all hide
                # under its 32 matmuls)
                qts[0] = qtpool.tile([P, NPAIR, SBLK], BF16, tag="qt", name="qt0")
                for f in range(FSH // P):
                    psq = pa_pool.tile([P, SBLK], F32, tag="pa")
                    for k in range(KSUB):
                        nc.tensor.matmul(
                            psq,
                            wqkv_sb[:, k, f * P : (f + 1) * P],
                            xt_sb[:, k, 0:SBLK],
                            start=(k == 0),
                            stop=(k == KSUB - 1),
                        )
                    if f == 0:
                        emit_b(bq.pop(0))
                    qt_elu(psq, 0, f)

                # extraction: blockdiag KV + replicated K_sum (zeros preset)
                for p_ in range(NPAIR):
                    nc.vector.tensor_copy(
                        out=lhsT2_sb[p_][0:HD, 0:HD], in_=kvps[p_][0:HD, 0:HD]
                    )
                    nc.vector.tensor_copy(
                        out=lhsT2_sb[p_][HD:P, HD:P], in_=kvps[p_][HD:P, HD:P]
                    )
                    nc.vector.tensor_copy(
                        out=ksumrep_sb[p_][0:HD, 0:HD],
                        in_=kvps[p_][0:HD, P : P + 1].to_broadcast((HD, HD)),
                    )
                    nc.vector.tensor_copy(
                        out=ksumrep_sb[p_][HD:P, HD:P],
                        in_=kvps[p_][HD:P, P : P + 1].to_broadcast((HD, HD)),
                    )

            # ---------------- phase 2: Q projection + attention + Wo -------
            with (
                tc.tile_pool(name="mm512", bufs=3, space="PSUM") as mmps,
                tc.tile_pool(name="pc", bufs=3, space="PSUM") as pcps,
                tc.tile_pool(name="pnb", bufs=2, space="PSUM") as pnps,
                tc.tile_pool(name="ou", bufs=3) as oupool,
                tc.tile_pool(name="rc", bufs=4) as rcpool,
                tc.tile_pool(name="ot", bufs=2) as otpool,
                tc.tile_pool(name="ys", bufs=2) as ypool,
            ):
                outus = {}
                rcbs = {}
                outts = {}

                def qt_half(j, fh):
                    if j not in qts:
                        qts[j] = qtpool.tile(
                            [P, NPAIR, SBLK], BF16, tag="qt", name=f"qt{j}"
                        )
                    for f in (2 * fh, 2 * fh + 1):
                        ps = mmps.tile([P, SBLK], F32, tag="mm")
                        for k in range(KSUB):
                            nc.tensor.matmul(
                                ps,
                                wqkv_sb[:, k, f * P : (f + 1) * P],
                                xt_sb[:, k, j * SBLK : (j + 1) * SBLK],
                                start=(k == 0),
                                stop=(k == KSUB - 1),
                            )
                        qt_elu(ps, j, f)

                def psc_section(j):
                    # per pair: attention matmul (ACT-evicted) + broadcast
                    # normalizer matmul (DVE fast reciprocal, stays in SBUF)
                    qtj = qts.pop(j)
                    outu = oupool.tile([P, NPAIR, SBLK], F32, tag="outu")
                    outus[j] = outu
                    rcbs[j] = []
                    for p_ in range(NPAIR):
                        psc = pcps.tile([P, SBLK], F32, tag="pc")
                        nc.tensor.matmul(
                            psc,
                            lhsT2_sb[p_],
                            qtj[:, p_, :],
                            start=True,
                            stop=True,
                        )
                        nc.scalar.copy(out=outu[:, p_, :], in_=psc)
                        psn = pnps.tile([P, SBLK], F32, tag="pn")
                        nc.tensor.matmul(
                            psn,
                            ksumrep_sb[p_],
                            qtj[:, p_, :],
                            start=True,
                            stop=True,
                        )
                        rcb = rcpool.tile([P, SBLK], F32, tag="rcb")
                        nc.vector.reciprocal_approx_fast(out=rcb[:], in_=psn[:])
                        rcbs[j].append(rcb)

                def mults(j):
                    outt = otpool.tile([P, NPAIR, SBLK], BF16, tag="outt")
                    outts[j] = outt
                    outu = outus.pop(j)
                    rcs = rcbs.pop(j)
                    for p_ in range(NPAIR):
                        nc.vector.tensor_tensor(
                            out=outt[:, p_, :],
                            in0=outu[:, p_, :],
                            in1=rcs[p_],
                            op=mybir.AluOpType.mult,
                        )

                def d_t(j, outt, t, drain=False):
                    ysb = ypool.tile([P, D], F32, tag="ysb", name="ysb")
                    psy0 = mmps.tile([P, 512], F32, tag="mm", name="psy0")
                    psy1 = mmps.tile([P, 512], F32, tag="mm", name="psy1")
                    for fs in range(FSH // P):
                        nc.tensor.matmul(
                            psy0,
                            outt[:, fs, t * P : (t + 1) * P],
                            wo_sb[:, fs, 0:512],
                            start=(fs == 0),
                            stop=(fs == FSH // P - 1),
                        )
                        nc.tensor.matmul(
                            psy1,
                            outt[:, fs, t * P : (t + 1) * P],
                            wo_sb[:, fs, 512:1024],
                            start=(fs == 0),
                            stop=(fs == FSH // P - 1),
                        )
                    nc.scalar.copy(out=ysb[:, 0:512], in_=psy0)
                    nc.sync.dma_start(out=y_rt[j, t, 0], in_=ysb[:, 0:512])
                    if drain:
                        # DVE is otherwise idle in the drain; parallel evict
                        nc.vector.tensor_copy(out=ysb[:, 512:1024], in_=psy1)
                    else:
                        nc.scalar.copy(out=ysb[:, 512:1024], in_=psy1)
                    nc.sync.dma_start(out=y_rt[j, t, 1], in_=ysb[:, 512:1024])

                def d_block(j):
                    outt = outts.pop(j)
                    for t in range(TSUB):
                        d_t(j, outt, t)

                def finale(j):
                    # drain block: apply-multiplies split per token subtile
                    # so each D chain starts as soon as its slice is scaled
                    outt = otpool.tile([P, NPAIR, SBLK], BF16, tag="outt")
                    outu = outus.pop(j)
                    rcs = rcbs.pop(j)
                    for t in range(TSUB):
                        sl = slice(t * P, (t + 1) * P)
                        for p_ in range(NPAIR):
                            nc.vector.tensor_tensor(
                                out=outt[:, p_, sl],
                                in0=outu[:, p_, sl],
                                in1=rcs[p_][:, sl],
                                op=mybir.AluOpType.mult,
                            )
                        d_t(j, outt, t, drain=(t >= TSUB - 2))

                # steady-state emission: block j's Q projection brackets
                # block j-1's attention chain so the PE never waits on the
                # ACT/DVE eviction+reciprocal+apply latency.
                for j in range(1, NBLK):
                    psc_section(j - 1)
                    mults(j - 1)
                    qt_half(j, 0)
                    qt_half(j, 1)
                    d_block(j - 1)
                psc_section(NBLK - 1)
                finale(NBLK - 1)

    nc.compile()
    return nc


def _prep_inputs(x, Wqkv, Wo):
    import ml_dtypes

    x = np.ascontiguousarray(x, dtype=np.float32)
    Wqkv = np.ascontiguousarray(Wqkv, dtype=np.float32)
    Wo = np.ascontiguousarray(Wo, dtype=np.float32)
    in_maps = []
    for b in range(B):
        xT = np.ascontiguousarray(x[b].T).astype(ml_dtypes.bfloat16)  # [D, S]
        for hh in range(2):
            cols = slice(hh * FSH, (hh + 1) * FSH)
            wq = Wqkv[:, 0 * D :][:, cols]
            wk = Wqkv[:, 1 * D :][:, cols]
            wv = Wqkv[:, 2 * D :][:, cols]
            wqkv_sh = np.ascontiguousarray(
                np.concatenate([wq, wk, wv], axis=1)
            ).astype(ml_dtypes.bfloat16)
            wo_sh = np.ascontiguousarray(Wo[hh * FSH : (hh + 1) * FSH, :]).astype(
                ml_dtypes.bfloat16
            )
            in_maps.append({"xT": xT, "wqkv": wqkv_sh, "wo": wo_sh})
    return in_maps


def kernel(x, Wqkv, Wo):
    global _NC_CACHE
    if _NC_CACHE is None:
        _NC_CACHE = build()
    nc = _NC_CACHE
    in_maps = _prep_inputs(x, Wqkv, Wo)
    res = run_bass_kernel_spmd(nc, in_maps, list(range(2 * B))).results
    y = np.empty((B, S, D), dtype=np.float32)
    for b in range(B):
        y[b] = res[2 * b]["y"] + res[2 * b + 1]["y"]
    return y


# revision 24
# speedup vs baseline: 1.0090x; 1.0090x over previous
"""Linear attention (B=4, S=4096, D=1024, H=16) on 8 TRN2 NeuronCores.

Sharding: core = (batch, head-half): each core handles one batch's 8 heads.
 - x is host-transposed to xT [D, S] per batch so both operand orientations
   of every matmul come out of the tensor engine with no on-device transpose.
 - Wqkv column-sharded per head-half; Wo row-sharded; host sums the two
   partial y's per batch (row-parallel unshard).

Two-phase dataflow (all matmuls bf16, fp32 PSUM accumulate):

phase 1 (per 512-token block): K,V projection token-major (lhsT=xT slice,
  rhs=Wk/Wv) -> elu+1(K) -> [KV | K_sum^T] PSUM accumulation per head-pair
  (vst carries a ones column so one matmul does both). Q is NOT computed
  here -- it is deferred to phase 2 so the PE has independent work to chew
  on across the KV -> attention transition (no pipeline bubble), and so x
  (kept resident in SBUF, 8MB bf16) is the only phase-1 input.
  Block 0 runs k-outer (4 simultaneous PSUM chains, one per 128-token
  subtile) so compute starts as soon as the first (wkv, x) DMA chunk lands
  instead of waiting for the full weight load.

phase 2 (per block, software-pipelined across j):
  QT [512f, 512s] feature-major (lhsT=Wq, rhs=xT slice) -> elu+1 -> bf16
  psc[128,s] = blockdiag(KV_h0, KV_h1)^T @ QT_pair: both heads of a pair
    in one matmul; ACT-evicted to outu
  norm: lhsT = [ksum_h0 replicated x64 | ksum_h1 replicated x64] so the
    matmul output IS the normalizer broadcast across all 128 partitions
    (no separate broadcast matmul); 1/x via the single-instruction DVE
    fast reciprocal (no Ln/Exp ACT ops, no activation-table switches)
  outT = outu * rcp (one DVE mult per pair, bf16)
  y[s,:] = outT^T @ Wo per 128-token subtile, fp32 out, DMAed per subtile
    (512KB chunks) to keep the drain tail short.
"""

import numpy as np

import concourse.bacc as bacc
import concourse.mybir as mybir
import concourse.tile as tile
from concourse.bass_utils import run_bass_kernel_spmd

F32 = mybir.dt.float32
BF16 = mybir.dt.bfloat16
ACT = mybir.ActivationFunctionType

P = 128
B, S, D = 4, 4096, 1024
H = 16
HD = 64

FSH = 512            # features per core for each of Q, K, V (8 heads)
KSUB = D // P        # 8 contraction subtiles
SBLK = 512           # tokens per block
NBLK = S // SBLK     # 8 blocks
TSUB = SBLK // P     # 4 token subtiles per block
NPAIR = 4            # head pairs per core

_NC_CACHE = None


def build():
    nc = bacc.Bacc(target_bir_lowering=False)
    xT = nc.dram_tensor("xT", [D, S], BF16, kind="ExternalInput")
    wqkv = nc.dram_tensor("wqkv", [D, 3 * FSH], BF16, kind="ExternalInput")
    wo = nc.dram_tensor("wo", [FSH, D], BF16, kind="ExternalInput")
    y = nc.dram_tensor("y", [S, D], F32, kind="ExternalOutput")

    xT_r = xT.rearrange("(ko p) s -> p ko s", p=P)        # [128, 8, 4096]
    wqkv_r = wqkv.rearrange("(ko p) f -> p ko f", p=P)    # [128, 8, 1536]
    wo_r = wo.rearrange("(fo p) n -> p fo n", p=P)        # [128, 4, 1024]
    y_rt = y.rearrange(
        "(j t p) (nh n) -> j t nh p n", t=TSUB, p=P, nh=2
    )  # [8,4,2,128,512]

    with tile.TileContext(nc) as tc:
        import contextlib

        with contextlib.ExitStack() as ctx:
            wpool = ctx.enter_context(tc.tile_pool(name="wpool", bufs=1))

            # persistent SBUF
            xt_sb = wpool.tile([P, KSUB, S], BF16)          # all of x, 64KB/p
            wqkv_sb = wpool.tile([P, KSUB, 3 * FSH], BF16)  # [wq|wk|wv]
            wo_sb = wpool.tile([P, FSH // P, D], BF16)
            # per-pair block-diagonal [[KV_h0, 0], [0, KV_h1]] (128x128)
            lhsT2_sb = [
                wpool.tile([P, P], BF16, name=f"l2{p}") for p in range(NPAIR)
            ]
            # per-pair [ksum_h0 x64 | ksum_h1 x64] replicated along free dim:
            # norm matmul output comes out already broadcast per head-half
            ksumrep_sb = [
                wpool.tile([P, P], BF16, name=f"kr{p}") for p in range(NPAIR)
            ]

            # Only block-0-critical transfers go on the sync queue (issued at
            # t=0): x block 0 + wv interleaved per contraction subtile, then
            # wk. Everything else (x blocks 1-7, wq, wo) is issued from the
            # scalar engine's DGE mid-block-0 — gated behind ops that already
            # depend on block-0 data — so it cannot steal HBM bandwidth from
            # the startup-critical path.
            for k in range(KSUB):
                nc.sync.dma_start(
                    out=xt_sb[:, k, 0:SBLK], in_=xT_r[:, k, 0:SBLK]
                )
                nc.sync.dma_start(
                    out=wqkv_sb[:, k, 2 * FSH : 3 * FSH],
                    in_=wqkv_r[:, k, 2 * FSH : 3 * FSH],
                )
            for k in range(KSUB):
                nc.sync.dma_start(
                    out=wqkv_sb[:, k, FSH : 2 * FSH],
                    in_=wqkv_r[:, k, FSH : 2 * FSH],
                )
            for p_ in range(NPAIR):
                nc.vector.memset(lhsT2_sb[p_], 0.0)
                nc.vector.memset(ksumrep_sb[p_], 0.0)

            # Non-critical DMAs queue behind the block-0 set on the same
            # hardware queue, so single-queue in-order dispatch gates them
            # without explicit dependencies.
            for j in range(1, NBLK):
                nc.sync.dma_start(
                    out=xt_sb[:, :, j * SBLK : (j + 1) * SBLK],
                    in_=xT_r[:, :, j * SBLK : (j + 1) * SBLK],
                )
            for k in range(KSUB):
                nc.sync.dma_start(
                    out=wqkv_sb[:, k, 0:FSH], in_=wqkv_r[:, k, 0:FSH]
                )
            nc.sync.dma_start(out=wo_sb, in_=wo_r)

            # SBUF pools shared across both phases
            etpool = ctx.enter_context(tc.tile_pool(name="et", bufs=3))
            qtpool = ctx.enter_context(tc.tile_pool(name="qt", bufs=2))
            qts = {}

            def qt_elu(ps, j, f):
                # elu(x)+1 = min(exp(x),1) + relu(x); Exp/Relu on ACT, the
                # combine on DVE
                e = etpool.tile([P, SBLK], F32, tag="e")
                nc.scalar.activation(out=e, in_=ps, func=ACT.Exp)
                r = etpool.tile([P, SBLK], F32, tag="r")
                nc.scalar.activation(out=r, in_=ps, func=ACT.Relu)
                nc.vector.scalar_tensor_tensor(
                    out=qts[j][:, f, :],
                    in0=e,
                    scalar=1.0,
                    in1=r,
                    op0=mybir.AluOpType.min,
                    op1=mybir.AluOpType.add,
                )

            # ---------------- phase 1: K,V projection + KV accumulation ----
            with (
                tc.tile_pool(name="kvps", bufs=1, space="PSUM") as kvps_pool,
                tc.tile_pool(name="pa", bufs=4, space="PSUM") as pa_pool,
                tc.tile_pool(name="st", bufs=2) as stpool,
            ):
                kvps = [
                    kvps_pool.tile([P, P + 1], F32, tag=f"kv{p}", name=f"kv{p}")
                    for p in range(NPAIR)
                ]

                bq = []  # lagged [KV | K_sum] accumulation entries

                def emit_b(ent):
                    kst, vst, j, t = ent
                    first = j == 0 and t == 0
                    last = j == NBLK - 1 and t == TSUB - 1
                    for p_ in range(NPAIR):
                        nc.tensor.matmul(
                            kvps[p_],
                            kst[:, t, p_ * P : (p_ + 1) * P],
                            vst[:, t, p_, :],
                            start=first,
                            stop=last,
                        )

                def elu_k(ps, kst, t):
                    e = etpool.tile([P, SBLK], F32, tag="e")
                    nc.scalar.activation(out=e, in_=ps, func=ACT.Exp)
                    r = etpool.tile([P, SBLK], F32, tag="r")
                    nc.vector.tensor_scalar_max(r, ps, 0.0)
                    nc.vector.scalar_tensor_tensor(
                        out=kst[:, t, :],
                        in0=e,
                        scalar=1.0,
                        in1=r,
                        op0=mybir.AluOpType.min,
                        op1=mybir.AluOpType.add,
                    )

                # block 0: V-sweep k-outer so PE work tracks DMA chunk
                # arrival (V needs no activation, so the K sweep that
                # follows runs at PE speed with elu pipelined per subtile)
                kst0 = stpool.tile([P, TSUB, FSH], BF16, tag="kst")
                vst0 = stpool.tile([P, TSUB, NPAIR, P + 1], BF16, tag="vst")
                nc.vector.memset(vst0[:, :, :, P : P + 1], 1.0)
                psvs = [
                    pa_pool.tile([P, SBLK], F32, tag="pa", name=f"psv{t}")
                    for t in range(TSUB)
                ]
                for k in range(KSUB):
                    for t in range(TSUB):
                        nc.tensor.matmul(
                            psvs[t],
                            xt_sb[:, k, t * P : (t + 1) * P],
                            wqkv_sb[:, k, 2 * FSH : 3 * FSH],
                            start=(k == 0),
                            stop=(k == KSUB - 1),
                        )
                for t in range(TSUB):
                    nc.scalar.copy(out=vst0[:, t, :, 0:P], in_=psvs[t])
                for t in range(TSUB):
                    psk = pa_pool.tile([P, SBLK], F32, tag="pa", name=f"psk{t}")
                    for k in range(KSUB):
                        nc.tensor.matmul(
                            psk,
                            xt_sb[:, k, t * P : (t + 1) * P],
                            wqkv_sb[:, k, FSH : 2 * FSH],
                            start=(k == 0),
                            stop=(k == KSUB - 1),
                        )
                    if t >= 1:
                        emit_b(bq.pop(0))
                    elu_k(psk, kst0, t)
                    bq.append((kst0, vst0, 0, t))

                # blocks 1..7: token-subtile-outer, B lagged one step
                for j in range(1, NBLK):
                    kst = stpool.tile([P, TSUB, FSH], BF16, tag="kst")
                    vst = stpool.tile([P, TSUB, NPAIR, P + 1], BF16, tag="vst")
                    nc.vector.memset(vst[:, :, :, P : P + 1], 1.0)
                    for t in range(TSUB):
                        tok = j * SBLK + t * P
                        psk = pa_pool.tile([P, SBLK], F32, tag="pa")
                        psv = pa_pool.tile([P, SBLK], F32, tag="pa")
                        for k in range(KSUB):
                            nc.tensor.matmul(
                                psk,
                                xt_sb[:, k, tok : tok + P],
                                wqkv_sb[:, k, FSH : 2 * FSH],
                                start=(k == 0),
                                stop=(k == KSUB - 1),
                            )
                            nc.tensor.matmul(
                                psv,
                                xt_sb[:, k, tok : tok + P],
                                wqkv_sb[:, k, 2 * FSH : 3 * FSH],
                                start=(k == 0),
                                stop=(k == KSUB - 1),
                            )
                        emit_b(bq.pop(0))
                        elu_k(psk, kst, t)
                        nc.scalar.copy(out=vst[:, t, :, 0:P], in_=psv)
                        bq.append((kst, vst, j, t))
                # block 0's Q projection runs here, inside the phase-1 PSUM
                # pools: it has no dependency on the KV state, so it keeps
                # the PE busy across the phase boundary (the trailing elu,
                # the KV extraction, and the phase-2 pool handover all hide
                # under its 32 matmuls)
                qts[0] = qtpool.tile([P, NPAIR, SBLK], BF16, tag="qt", name="qt0")
                for f in range(FSH // P):
                    psq = pa_pool.tile([P, SBLK], F32, tag="pa")
                    for k in range(KSUB):
                        nc.tensor.matmul(
                            psq,
                            wqkv_sb[:, k, f * P : (f + 1) * P],
                            xt_sb[:, k, 0:SBLK],
                            start=(k == 0),
                            stop=(k == KSUB - 1),
                        )
                    if f == 0:
                        emit_b(bq.pop(0))
                    qt_elu(psq, 0, f)

                # extraction: blockdiag KV + replicated K_sum (zeros preset)
                for p_ in range(NPAIR):
                    nc.vector.tensor_copy(
                        out=lhsT2_sb[p_][0:HD, 0:HD], in_=kvps[p_][0:HD, 0:HD]
                    )
                    nc.vector.tensor_copy(
                        out=lhsT2_sb[p_][HD:P, HD:P], in_=kvps[p_][HD:P, HD:P]
                    )
                    nc.vector.tensor_copy(
                        out=ksumrep_sb[p_][0:HD, 0:HD],
                        in_=kvps[p_][0:HD, P : P + 1].to_broadcast((HD, HD)),
                    )
                    nc.vector.tensor_copy(
                        out=ksumrep_sb[p_][HD:P, HD:P],
                        in_=kvps[p_][HD:P, P : P + 1].to_broadcast((HD, HD)),
                    )

            # ---------------- phase 2: Q projection + attention + Wo -------
            with (
                tc.tile_pool(name="mm512", bufs=3, space="PSUM") as mmps,
                tc.tile_pool(name="pc", bufs=3, space="PSUM") as pcps,
                tc.tile_pool(name="pnb", bufs=2, space="PSUM") as pnps,
                tc.tile_pool(name="ou", bufs=3) as oupool,
                tc.tile_pool(name="rc", bufs=4) as rcpool,
                tc.tile_pool(name="ot", bufs=2) as otpool,
                tc.tile_pool(name="ys", bufs=2) as ypool,
            ):
                outus = {}
                rcbs = {}
                outts = {}

                def qt_half(j, fh):
                    if j not in qts:
                        qts[j] = qtpool.tile(
                            [P, NPAIR, SBLK], BF16, tag="qt", name=f"qt{j}"
                        )
                    for f in (2 * fh, 2 * fh + 1):
                        ps = mmps.tile([P, SBLK], F32, tag="mm")
                        for k in range(KSUB):
                            nc.tensor.matmul(
                                ps,
                                wqkv_sb[:, k, f * P : (f + 1) * P],
                                xt_sb[:, k, j * SBLK : (j + 1) * SBLK],
                                start=(k == 0),
                                stop=(k == KSUB - 1),
                            )
                        qt_elu(ps, j, f)

                def psc_section(j):
                    # per pair: attention matmul (ACT-evicted) + broadcast
                    # normalizer matmul (DVE fast reciprocal, stays in SBUF)
                    qtj = qts.pop(j)
                    outu = oupool.tile([P, NPAIR, SBLK], F32, tag="outu")
                    outus[j] = outu
                    rcbs[j] = []
                    for p_ in range(NPAIR):
                        psc = pcps.tile([P, SBLK], F32, tag="pc")
                        nc.tensor.matmul(
                            psc,
                            lhsT2_sb[p_],
                            qtj[:, p_, :],
                            start=True,
                            stop=True,
                        )
                        nc.scalar.copy(out=outu[:, p_, :], in_=psc)
                        psn = pnps.tile([P, SBLK], F32, tag="pn")
                        nc.tensor.matmul(
                            psn,
                            ksumrep_sb[p_],
                            qtj[:, p_, :],
                            start=True,
                            stop=True,
                        )
                        rcb = rcpool.tile([P, SBLK], F32, tag="rcb")
                        nc.vector.reciprocal_approx_fast(out=rcb[:], in_=psn[:])
                        rcbs[j].append(rcb)

                def mults(j):
                    outt = otpool.tile([P, NPAIR, SBLK], BF16, tag="outt")
                    outts[j] = outt
                    outu = outus.pop(j)
                    rcs = rcbs.pop(j)
                    for p_ in range(NPAIR):
                        nc.vector.tensor_tensor(
                            out=outt[:, p_, :],
                            in0=outu[:, p_, :],
                            in1=rcs[p_],
                            op=mybir.AluOpType.mult,
                        )

                def d_t(j, outt, t, drain=False):
                    ysb = ypool.tile([P, D], F32, tag="ysb", name="ysb")
                    psy0 = mmps.tile([P, 512], F32, tag="mm", name="psy0")
                    psy1 = mmps.tile([P, 512], F32, tag="mm", name="psy1")
                    for fs in range(FSH // P):
                        nc.tensor.matmul(
                            psy0,
                            outt[:, fs, t * P : (t + 1) * P],
                            wo_sb[:, fs, 0:512],
                            start=(fs == 0),
                            stop=(fs == FSH // P - 1),
                        )
                        nc.tensor.matmul(
                            psy1,
                            outt[:, fs, t * P : (t + 1) * P],
                            wo_sb[:, fs, 512:1024],
                            start=(fs == 0),
                            stop=(fs == FSH // P - 1),
                        )
                    nc.scalar.copy(out=ysb[:, 0:512], in_=psy0)
                    nc.sync.dma_start(out=y_rt[j, t, 0], in_=ysb[:, 0:512])
                    if drain:
                        # DVE is otherwise idle in the drain; parallel evict
                        nc.vector.tensor_copy(out=ysb[:, 512:1024], in_=psy1)
                    else:
                        nc.scalar.copy(out=ysb[:, 512:1024], in_=psy1)
                    nc.sync.dma_start(out=y_rt[j, t, 1], in_=ysb[:, 512:1024])

                def d_block(j):
                    outt = outts.pop(j)
                    for t in range(TSUB):
                        d_t(j, outt, t)

                def finale(j):
                    # drain block: apply-multiplies split per token subtile
                    # so each D chain starts as soon as its slice is scaled
                    outt = otpool.tile([P, NPAIR, SBLK], BF16, tag="outt")
                    outu = outus.pop(j)
                    rcs = rcbs.pop(j)
                    for t in range(TSUB):
                        sl = slice(t * P, (t + 1) * P)
                        for p_ in range(NPAIR):
                            nc.vector.tensor_tensor(
                                out=outt[:, p_, sl],
                                in0=outu[:, p_, sl],
                                in1=rcs[p_][:, sl],
                                op=mybir.AluOpType.mult,
                            )
                        d_t(j, outt, t, drain=(t == TSUB - 1))

                # steady-state emission: block j's Q projection brackets
                # block j-1's attention chain so the PE never waits on the
                # ACT/DVE eviction+reciprocal+apply latency.
                for j in range(1, NBLK):
                    psc_section(j - 1)
                    mults(j - 1)
                    qt_half(j, 0)
                    qt_half(j, 1)
                    d_block(j - 1)
                psc_section(NBLK - 1)
                finale(NBLK - 1)

    nc.compile()
    return nc


def _prep_inputs(x, Wqkv, Wo):
    import ml_dtypes

    x = np.ascontiguousarray(x, dtype=np.float32)
    Wqkv = np.ascontiguousarray(Wqkv, dtype=np.float32)
    Wo = np.ascontiguousarray(Wo, dtype=np.float32)
    in_maps = []
    for b in range(B):
        xT = np.ascontiguousarray(x[b].T).astype(ml_dtypes.bfloat16)  # [D, S]
        for hh in range(2):
            cols = slice(hh * FSH, (hh + 1) * FSH)
            wq = Wqkv[:, 0 * D :][:, cols]
            wk = Wqkv[:, 1 * D :][:, cols]
            wv = Wqkv[:, 2 * D :][:, cols]
            wqkv_sh = np.ascontiguousarray(
                np.concatenate([wq, wk, wv], axis=1)
            ).astype(ml_dtypes.bfloat16)
            wo_sh = np.ascontiguousarray(Wo[hh * FSH : (hh + 1) * FSH, :]).astype(
                ml_dtypes.bfloat16
            )
            in_maps.append({"xT": xT, "wqkv": wqkv_sh, "wo": wo_sh})
    return in_maps


def kernel(x, Wqkv, Wo):
    global _NC_CACHE
    if _NC_CACHE is None:
        _NC_CACHE = build()
    nc = _NC_CACHE
    in_maps = _prep_inputs(x, Wqkv, Wo)
    res = run_bass_kernel_spmd(nc, in_maps, list(range(2 * B))).results
    y = np.empty((B, S, D), dtype=np.float32)
    for b in range(B):
        y[b] = res[2 * b]["y"] + res[2 * b + 1]["y"]
    return y
